# revision 5
# baseline (speedup 1.0000x reference)
"""Trainium2 Bass kernel for the CachedMPS classifier (nn_CachedMPS_68212670595935).

Matrix Product State classifier over N=784 sites, D=64 bond dim, batch 8192:
  feats = (cos(pi/2 x), sin(pi/2 x));  M0 = feats[:,0,:] @ core0
  scan 782 mid sites: M <- l2norm(M @ (c0*A0 + c1*A1));  logits = last-site contraction

Per-step L2 norms are per-row scales that commute with the linear step maps, so
they cancel in one final normalization (eps=1e-8 effects are ~1e-8 relative).
The device scan runs un-normalized with a constant alpha folded into the mid
cores for fp32 range control, and normalizes once at the end.

v2: the profile of the v1 kernel showed the tensor engine 97% busy: 4 matmuls
per site (2 state + 2 feature-replication) at ~530ns each. This version keeps
only the 2 state matmuls on PE:
  - features are computed host-side and shipped as fcrep [782, 2, Bs] fp16 in
    DRAM; the per-site [128, Bs] replicated feature tile (crep) is produced by
    broadcast DMAs (stride-0 leading dim on the DRAM source AP), 4 sites per
    issue, so neither PE (replication matmuls) nor ACT (PSUM->SBUF copies) do
    any crep work anymore.
  - the site-0 init state V1 = [c0(1)*M0; c1(1)*M0]^T is computed on host.
  - weights stream in groups of 8 sites per DMA with 2-group lookahead.
Steady state per site: PE 2x512-col fp32r MMs (~850ns @1.2GHz), DVE 2 TTs
(~1370ns) -> DVE-bound at ~1.4us/site vs 2.17us/site for v1.
"""

import numpy as np
from contextlib import ExitStack

import concourse.bass as bass
import concourse.tile as tile
from concourse import bacc, mybir
from concourse.bass_utils import run_bass_kernel_spmd

F32 = mybir.dt.float32
F32R = mybir.dt.float32r
F16 = mybir.dt.float16
AF = mybir.ActivationFunctionType

D = 64
C = 10
N_CORES = 8
HALF_PI = float(np.pi / 2.0)
ALPHA = float(2.0 ** (-1.0 / 3.0))
WG = 8   # sites per weight-DMA group
CG = 4   # sites per crep-broadcast group


def build_nc(n_sites: int, Bs: int):
    n_mid = n_sites - 2            # 782 scan steps
    chunk = 512
    n_chunks = Bs // chunk
    assert Bs % chunk == 0

    nc = bacc.Bacc("TRN2", target_bir_lowering=False, debug=False)

    V1d = nc.dram_tensor("V1", [128, Bs], F32R, kind="ExternalInput")
    Ad = nc.dram_tensor("Ad", [128, n_mid, 128], F32R, kind="ExternalInput")
    Fc = nc.dram_tensor("Fc", [n_mid, 2, Bs], F16, kind="ExternalInput")
    WL = nc.dram_tensor("WL", [128, C], F32R, kind="ExternalInput")
    ones64 = nc.dram_tensor("ones64", [D, 1], F32R, kind="ExternalInput")
    ones10 = nc.dram_tensor("ones10", [1, C], F32R, kind="ExternalInput")
    out = nc.dram_tensor("out", [C, Bs], F32, kind="ExternalOutput")

    with tile.TileContext(nc) as tc:
        with ExitStack() as ctx:
            const = ctx.enter_context(tc.tile_pool(name="const", bufs=1))
            vpool = ctx.enter_context(tc.tile_pool(name="vpool", bufs=2))
            wpool = ctx.enter_context(tc.tile_pool(name="wpool", bufs=3))
            crpool = ctx.enter_context(tc.tile_pool(name="crpool", bufs=4))
            endp = ctx.enter_context(tc.tile_pool(name="endp", bufs=1))
            pp = ctx.enter_context(tc.tile_pool(name="pp", bufs=2, space="PSUM"))
            cp = ctx.enter_context(tc.tile_pool(name="cp", bufs=2, space="PSUM"))

            wl_sb = const.tile([128, C], F32R)
            nc.sync.dma_start(wl_sb[:], WL.ap())
            o64_sb = const.tile([D, 1], F32R)
            nc.sync.dma_start(o64_sb[:], ones64.ap())
            o10_sb = const.tile([1, C], F32R)
            nc.sync.dma_start(o10_sb[:], ones10.ap())

            # ---- streamed weight groups: [128, WG, 128] per group
            def w_group(g):
                s0 = g * WG
                k = min(WG, n_mid - s0)
                wt = wpool.tile([128, WG, 128], F32R, tag="w")
                nc.sync.dma_start(wt[:, 0:k, :], Ad.ap()[:, s0:s0 + k, :])
                return wt

            # ---- crep groups: broadcast DMA from DRAM features
            # crep[p, i, b] = Fc[s0+i, p//64, b]; leading stride-0 dim
            # replicates each feature row across 64 partitions.
            def c_group(g):
                s0 = g * CG
                k = min(CG, n_mid - s0)
                ct = crpool.tile([128, CG, Bs], F16, tag="c")
                # issue the two halves from the idle ACT/GPSIMD DGEs so the
                # broadcast traffic doesn't serialize on SP's dynamic queue
                nc.scalar.dma_start(ct[0:64, 0:k, :],
                                    Fc.ap()[s0:s0 + k, 0, :].partition_broadcast(64))
                nc.gpsimd.dma_start(ct[64:128, 0:k, :],
                                    Fc.ap()[s0:s0 + k, 1, :].partition_broadcast(64))
                return ct

            n_wg = (n_mid + WG - 1) // WG
            n_cg = (n_mid + CG - 1) // CG
            w_tiles = {g: w_group(g) for g in range(min(2, n_wg))}
            c_tiles = {g: c_group(g) for g in range(min(3, n_cg))}

            # ---- initial state
            v = []
            for c in range(n_chunks):
                vc = vpool.tile([128, chunk], F32R, tag=f"v{c}")
                nc.sync.dma_start(vc[:], V1d.ap()[:, c * chunk:(c + 1) * chunk])
                v.append(vc)

            # ---- main scan: step j applies mid-core j and features of site j+2
            wt = ct = None
            for j in range(n_mid):
                gw, iw = divmod(j, WG)
                if iw == 0:
                    wt = w_tiles.pop(gw)
                    if gw + 2 < n_wg and gw + 2 not in w_tiles:
                        w_tiles[gw + 2] = w_group(gw + 2)
                gc, ic = divmod(j, CG)
                if ic == 0:
                    ct = c_tiles.pop(gc)
                    if gc + 3 < n_cg and gc + 3 not in c_tiles:
                        c_tiles[gc + 3] = c_group(gc + 3)

                p2 = []
                for c in range(n_chunks):
                    pc = pp.tile([128, chunk], F32, tag=f"p2{c}")
                    nc.tensor.matmul(pc[:], wt[:, iw, :], v[c][:],
                                     start=True, stop=True)
                    p2.append(pc)
                vn = []
                for c in range(n_chunks):
                    vc = vpool.tile([128, chunk], F32R, tag=f"v{c}")
                    nc.vector.tensor_mul(vc[:], p2[c][:],
                                         ct[:, ic, c * chunk:(c + 1) * chunk])
                    vn.append(vc)
                v = vn
                last_p2 = p2

            # ---- endgame: logits + final normalization
            lg = cp.tile([C, Bs], F32, tag="eg")
            for c in range(n_chunks):
                nc.tensor.matmul(lg[:, c * chunk:(c + 1) * chunk], wl_sb[:],
                                 v[c][:], start=True, stop=True)
            sq = endp.tile([D, Bs], F32R)
            for c in range(n_chunks):
                nc.scalar.activation(sq[:, c * chunk:(c + 1) * chunk],
                                     last_p2[c][0:D, :], AF.Square)
            ns = cp.tile([1, Bs], F32, tag="eg")
            for c in range(n_chunks):
                nc.tensor.matmul(ns[:, c * chunk:(c + 1) * chunk], o64_sb[:],
                                 sq[:, c * chunk:(c + 1) * chunk],
                                 start=True, stop=True)
            rec = endp.tile([1, Bs], F32)
            rscr = endp.tile([1, Bs], F32)
            nc.vector.reciprocal_approx_accurate(rec[:], ns[:], rscr[:])
            inv = endp.tile([1, Bs], F32R)
            nc.scalar.activation(inv[:], rec[:], AF.Sqrt)
            irep = cp.tile([C, Bs], F32, tag="eg")
            for c in range(n_chunks):
                nc.tensor.matmul(irep[:, c * chunk:(c + 1) * chunk], o10_sb[:],
                                 inv[:, c * chunk:(c + 1) * chunk],
                                 start=True, stop=True)
            isb = endp.tile([C, Bs], F32)
            nc.scalar.copy(isb[:], irep[:])
            res = endp.tile([C, Bs], F32)
            nc.vector.tensor_mul(res[:], lg[:], isb[:])
            nc.sync.dma_start(out.ap(), res[:])

    nc.compile()
    return nc


def host_prep(x, core0, cores_mid, core_last, n_cores=N_CORES):
    x = np.asarray(x, np.float32)
    core0 = np.asarray(core0, np.float32)
    cores_mid = np.asarray(cores_mid, np.float32)
    core_last = np.asarray(core_last, np.float32)
    B, n_sites = x.shape
    n_mid = n_sites - 2
    Bs = B // n_cores

    th = HALF_PI * x
    fcos = np.cos(th)
    fsin = np.sin(th)

    M0 = fcos[:, 0:1] * core0[0, 0][None, :] + fsin[:, 0:1] * core0[1, 0][None, :]
    # V1[b] = [c0(site1)*M0 ; c1(site1)*M0]
    V1 = np.concatenate([fcos[:, 1:2] * M0, fsin[:, 1:2] * M0], axis=1)  # [B, 128]

    Aaug = (ALPHA * cores_mid).reshape(n_mid, 2 * D, D)
    Ad = np.concatenate([Aaug, Aaug], axis=2)            # [n_mid, 128, 128]
    Ad_t = np.ascontiguousarray(Ad.transpose(1, 0, 2))   # [128, n_mid, 128]
    WL = np.ascontiguousarray(core_last.reshape(2 * D, C), np.float32)
    ones64 = np.ones((D, 1), np.float32)
    ones10 = np.ones((1, C), np.float32)

    # features of sites 2..n_sites-1, [n_mid, 2, B] fp16
    fc_all = np.stack([fcos[:, 2:].T, fsin[:, 2:].T], axis=1).astype(np.float16)

    in_maps = []
    for c in range(n_cores):
        sl = slice(c * Bs, (c + 1) * Bs)
        in_maps.append({
            "V1": np.ascontiguousarray(V1[sl].T),
            "Ad": Ad_t,
            "Fc": np.ascontiguousarray(fc_all[:, :, sl]),
            "WL": WL, "ones64": ones64, "ones10": ones10,
        })
    return in_maps, Bs


_CACHE = {}


def _get_nc(n_sites, Bs):
    key = (n_sites, Bs)
    if key not in _CACHE:
        _CACHE[key] = build_nc(n_sites, Bs)
    return _CACHE[key]


def run(x, core0, cores_mid, core_last, trace=False, **kw):
    B, n_sites = np.asarray(x).shape
    in_maps, Bs = host_prep(x, core0, cores_mid, core_last)
    nc = _get_nc(n_sites, Bs)
    res = run_bass_kernel_spmd(nc, in_maps, core_ids=list(range(N_CORES)), trace=trace, **kw)
    logits = np.concatenate([r["out"].T for r in res.results], axis=0).astype(np.float32)
    return logits, res


def kernel(x, core0, cores_mid, core_last):
    logits, _ = run(x, core0, cores_mid, core_last)
    return logits
